# revision 1
# baseline (speedup 1.0000x reference)
"""TRN2 Bass kernel for nn_EvolutionModel_91173565759692 (self-contained).

Physics: 16384 rays, 100-step velocity-Verlet in ior-center-centered coords
  y_{t+1} = (2 + P(g))*y_t - y_{t-1},  g = exp(-2|y|^2), P = c1 g + c2 g^2
Sampling: exact per-ray searchsorted via a bucket LUT (width 2^-6) built with
GPSIMD local_scatter + DVE prefix scans; bracket payloads delivered to sample
slots by scatter + backward positional fills (TRN2 has no per-partition
gather).  8-way data-parallel over rays (2048 rays/core).
"""
import sys
sys.path.insert(0, "/opt/trn_rl_repo")
import numpy as np
import concourse.bass as bass
import concourse.bacc as bacc
import concourse.mybir as mybir
from concourse.tile import TileContext
import concourse.dve_ops as dve_ops
from concourse import dve_spec
from concourse.dve_spec import Spec, Src0, Src1, C0, C1, C2, One, sq, lower
from concourse.dve_uop import DveOpSpec
from concourse.dve_table_gen import dve_ver_for
from concourse.bass_utils import run_bass_kernel_spmd

f32 = mybir.dt.float32
i16 = mybir.dt.int16
u16 = mybir.dt.uint16
i32 = mybir.dt.int32
AF = mybir.ActivationFunctionType
ALU = mybir.AluOpType

N_STEPS = 100
SIGMA2x2 = 0.5
DT = np.float32(0.02)
KC = -DT * DT / np.float32(SIGMA2x2 / 2.0)

_registered = {}


def register_op(name, spec, subdim=False):
    if name in _registered:
        return _registered[name]
    ver = dve_ver_for("TRN2")
    row = dve_ops._CUSTOM_DVE_ROW_BASE + len(dve_ops.OPS)
    assert row < 0x20
    dve_ops._SUB_OPCODE_FOR_NAME[name] = row
    tmp = DveOpSpec(name=name, opcode=row, uops=lower(spec, ver=ver),
                    rd1_en=dve_spec._has_src1(spec))
    op = dve_ops.DveOp(name, spec, subdim, {ver: tmp.sha(ver)})
    dve_ops.OPS.append(op)
    dve_ops.CUSTOM_DVE_SPECS[name] = spec
    _registered[name] = op
    return op


# W = (g*C0 + C1)*g + imm2   (C0=c2 tile, C1=c1 tile, imm2 = 2 or 1)
OP_WPOLY = lambda: register_op(
    "ANT_EVO_WPOLY",
    Spec(body=(Src0 * C0 + C1) * Src0 + C2,
         reference=lambda in0, in1, s0, s1, imm2: (
             in0.astype(np.float32) * s0 + s1) * in0 + imm2),
)

# out = (Src0 - Src1)^2
OP_SUBSQ = lambda: register_op(
    "ANT_EVO_SUBSQ",
    Spec(body=sq(Src0 - Src1),
         reference=lambda in0, in1, s0, s1, imm2: (
             (in0.astype(np.float32) - in1) ** 2)),
)

# out = Src0*Src0 + Src1*Src1
OP_SQ2 = lambda: register_op(
    "ANT_EVO_SQ2",
    Spec(body=Src0 * Src0 + Src1 * Src1,
         reference=lambda in0, in1, s0, s1, imm2: (
             in0.astype(np.float32) ** 2 + in1.astype(np.float32) ** 2)),
)

# out = Src0*Src0 + Src1   (square-accumulate)
OP_SQA = lambda: register_op(
    "ANT_EVO_SQA",
    Spec(body=Src0 * Src0 + Src1,
         reference=lambda in0, in1, s0, s1, imm2: (
             in0.astype(np.float32) ** 2 + in1.astype(np.float32))),
)

# P1: b = (((x*C0 + C1)*x + C2)*x + Src1)*x   (x = Src0 = r2; Src1 = coef c2 bcast)
OP_EXP_P1 = lambda: register_op(
    "ANT_EVO_EXP_P1",
    Spec(body=(((Src0 * C0 + C1) * Src0 + C2) * Src0 + Src1) * Src0,
         reference=lambda in0, in1, s0, s1, imm2: (
             (((in0.astype(np.float32) * s0 + s1) * in0 + imm2) * in0 + in1) * in0)),
)

# P2: u = (Src0 + C0)*Src1 + C1 ; out = u^32  (Src0 = b, Src1 = x = r2)
def _p2_ref(in0, in1, s0, s1, imm2):
    u = ((in0.astype(np.float32) + s0) * in1 + s1)
    for _ in range(5):
        u = u * u
    return u

def _p2_body():
    u = (Src0 + C0) * Src1 + C1
    for _ in range(5):
        u = sq(u)
    return u

OP_EXP_P2 = lambda: register_op(
    "ANT_EVO_EXP_P2", Spec(body=_p2_body(), reference=_p2_ref))

# P3: g = Src0^2 ; W = (g*C0 + C1)*g + C2
def _p3_ref(in0, in1, s0, s1, imm2):
    g = in0.astype(np.float32) ** 2
    return (g * s0 + s1) * g + imm2

def _p3_body():
    g = sq(Src0)
    return (g * C0 + C1) * g + C2

OP_EXP_P3 = lambda: register_op(
    "ANT_EVO_EXP_P3", Spec(body=_p3_body(), reference=_p3_ref))


def fit_exp_poly():
    """Tail-weighted deg-5 fit: u(r2) ~= exp(-r2/16) on r2 in [0,27], tight on
    [0,12] (where g=u^32 >= ~1e-8 matters); loose tail out to r2=32. Returns c[0..5] in r2 powers."""
    xs_t = -0.75 * (np.cos(np.linspace(0, np.pi, 3000)) * 0.5 + 0.5)
    xs_l = np.linspace(-2.0, -0.75, 1200)
    x = np.concatenate([xs_t, xs_l]); y = np.exp(x)
    w = np.where(x >= -0.75, 1.0 / y, 0.02 / y)
    V = np.vander(x, 6)
    coef = np.linalg.lstsq(V * w[:, None], y * w, rcond=None)[0]
    c = coef[::-1].copy()
    sc = np.array([(-1.0 / 16.0) ** i for i in range(6)])
    return (c * sc).astype(np.float64)


# QW: q = Src0 * ((Src1*C0 + C1)*Src1 + k)   (Src0=y stream, Src1=g bcast,
#   C0=c2 tile, C1=c1 tile, k=2 (interior) or 1 (first step) via One leaves)
def _qw2_ref(in0, in1, s0, s1, imm2):
    return in0.astype(np.float32) * ((in1 * s0 + s1) * in1 + 2.0)

def _qw1_ref(in0, in1, s0, s1, imm2):
    return in0.astype(np.float32) * ((in1 * s0 + s1) * in1 + 1.0)

OP_QW2 = lambda: register_op(
    "ANT_EVO_QW2",
    Spec(body=Src0 * ((Src1 * C0 + C1) * Src1 + One + One), reference=_qw2_ref))

OP_QW1 = lambda: register_op(
    "ANT_EVO_QW1",
    Spec(body=Src0 * ((Src1 * C0 + C1) * Src1 + One), reference=_qw1_ref))


def build_integration(nc, tc, pool, x0c, v0c, A, cvec, H, Dh,
                      with_distances=True, mode="alldve", groups=2):
    """Emit integration. x0c/v0c: DRAM [128,48]; A, cvec: python floats
    (ior_amp scalar, ior_center 3-vector) baked at build time.
    H: SBUF tile [128, 101, 48]; Dh: SBUF tile [128, 16, 101]."""
    v = nc.vector
    s = nc.scalar
    subsq = OP_SUBSQ()
    sq2 = OP_SQ2()
    sqa = OP_SQA()

    A = float(np.float32(A))
    c1f = float(np.float32(KC) * np.float32(A))
    c2f = float(np.float32(c1f) * np.float32(A))
    c1hf = float(np.float32(c1f) * np.float32(0.5))
    c2hf = float(np.float32(c2f) * np.float32(0.5))

    # --- load & prep
    x0t = pool.tile([128, 48], f32)
    nc.sync.dma_start(x0t[:, :], x0c[:, :])
    u0 = pool.tile([128, 48], f32)
    nc.sync.dma_start(u0[:, :], v0c[:, :])
    v.tensor_scalar_mul(u0[:, :], u0[:, :], float(DT))  # u0 = dt*v0

    H3 = H  # [128, 101, 48]

    # y0 = x0 - c  -> hist[0]  (3 strided per-component subtracts)
    x03 = x0t[:, :].rearrange("p (a c) -> p a c", c=3)
    h03 = H3[:, 0, :].rearrange("p (a c) -> p a c", c=3)
    for ci in range(3):
        v.tensor_scalar_add(h03[:, :, ci], x03[:, :, ci], -float(np.float32(cvec[ci])))

    qw2 = OP_QW2()
    qw1 = OP_QW1()
    p1 = OP_EXP_P1()
    p2 = OP_EXP_P2()
    pc = fit_exp_poly()
    GR = groups if mode == "alldve" else groups
    gs = 16 // GR       # ray slots per group
    merged_poly = False
    t12s = [pool.tile([128, gs], f32, name=f"t12_{gi}") for gi in range(GR)]
    r2all = pool.tile([128, 16], f32, name="r2all")
    r2s = [r2all[:, gi * gs:(gi + 1) * gs] for gi in range(GR)]
    ball = pool.tile([128, 16], f32, name="ball")
    bts = [ball[:, gi * gs:(gi + 1) * gs] for gi in range(GR)]
    gall = pool.tile([128, 16], f32, name="gall")
    gts = [gall[:, gi * gs:(gi + 1) * gs] for gi in range(GR)]
    qall = pool.tile([128, 48], f32, name="qall")
    qs = [qall[:, gi * gs * 3:(gi + 1) * gs * 3] for gi in range(GR)]
    c2coef = pool.tile([128, 1], f32, name="c2coef")
    v.memset(c2coef[:, :], float(pc[2]))
    c2cb = c2coef[:, :].to_broadcast([128, gs])
    c2cb16 = c2coef[:, :].to_broadcast([128, 16])

    def yv(t, gi):  # [128, gs, 3] view of hist at step t, group gi
        return H3[:, t, gi * gs * 3:(gi + 1) * gs * 3].rearrange(
            "p (a c) -> p a c", c=3)

    def gcalc(t, gi):
        # r2 = |y|^2 -> g = exp(-2 r2)
        y3 = yv(t, gi)
        v._custom_dve(sq2, out=t12s[gi][:, :], in0=y3[:, :, 0], in1=y3[:, :, 1])
        v._custom_dve(sqa, out=r2s[gi], in0=y3[:, :, 2], in1=t12s[gi][:, :])
        if mode == "alldve" and not merged_poly:
            v._custom_dve(p1, out=bts[gi], in0=r2s[gi], in1=c2cb,
                          s0=float(pc[5]), s1=float(pc[4]), imm2=float(pc[3]))
            v._custom_dve(p2, out=gts[gi], in0=bts[gi],
                          in1=r2s[gi], s0=float(pc[1]), s1=float(pc[0]))
        elif mode != "alldve":
            s.activation(gts[gi], r2s[gi], AF.Exp, scale=-2.0)

    def polycalc():
        # merged deg-5 poly + ^32 over all 16 ray-slots
        v._custom_dve(p1, out=ball[:, :], in0=r2all[:, :], in1=c2cb16,
                      s0=float(pc[5]), s1=float(pc[4]), imm2=float(pc[3]))
        v._custom_dve(p2, out=gall[:, :], in0=ball[:, :], in1=r2all[:, :],
                      s0=float(pc[1]), s1=float(pc[0]))

    def qcalc(t, gi, c1x, c2x, op):
        # q = y_t * ((g*c2 + c1)*g + k)   (c1x/c2x compile-time floats)
        gb = gts[gi].rearrange("p (a o) -> p a o", o=1).to_broadcast(
            [128, gs, 3])
        v._custom_dve(op, out=qs[gi].rearrange("p (a c) -> p a c", c=3),
                      in0=yv(t, gi), in1=gb, s0=c2x, s1=c1x)

    gsl = lambda gi: slice(gi * gs * 3, (gi + 1) * gs * 3)

    # first step: y1 = (1 + P/2)*y0 + u0
    for gi in range(GR):
        gcalc(0, gi)
    if merged_poly:
        polycalc()
    for gi in range(GR):
        qcalc(0, gi, c1hf, c2hf, qw1)
        v.tensor_tensor(H3[:, 1, gsl(gi)], qs[gi], u0[:, gsl(gi)], ALU.add)

    # interior steps: y_{t+1} = (2 + P)*y_t - y_{t-1}
    # emission order pipelines groups across DVE/ACT
    merged_ynext = False
    for t in range(1, N_STEPS):
        for gi in range(GR):
            gcalc(t, gi)
        if merged_poly:
            polycalc()
        for gi in range(GR):
            qcalc(t, gi, c1f, c2f, qw2)
            if not merged_ynext:
                v.tensor_tensor(H3[:, t + 1, gsl(gi)], qs[gi],
                                H3[:, t - 1, gsl(gi)], ALU.subtract)
        if merged_ynext:
            v.tensor_tensor(H3[:, t + 1, :], qall[:, :],
                            H3[:, t - 1, :], ALU.subtract)

    if not with_distances:
        return dict()

    # --- distances (transients in a scoped pool) ---
    dctx = tc.tile_pool(name="dist_scr", bufs=1)
    dpool = dctx.__enter__()
    dsq = dpool.tile([128, 1600, 3], f32)
    v._custom_dve(subsq, out=dsq[:, :, :],
                  in0=H3[:, 1:101, :].rearrange("p a (b c) -> p (a b) c", c=3),
                  in1=H3[:, 0:100, :].rearrange("p a (b c) -> p (a b) c", c=3))
    d2e = dpool.tile([128, 16, 101], f32)
    v.memset(d2e[:, :, 0:1], 0.0)
    # out iteration order must match input (t outer, ray inner): "p b a"
    v.tensor_reduce(
        d2e[:, :, 1:101].rearrange("p a b -> p b a"),
        dsq[:, :, :],
        axis=mybir.AxisListType.X, op=ALU.add)
    # d = sqrt(d2) (in place, slots 1..100)
    s.activation(d2e[:, :, 1:101], d2e[:, :, 1:101], AF.Sqrt)
    # Dh = per-ray cumsum over 101 slots (slot0 stays 0 since mask=0, d=0 there)
    mks = dpool.tile([128, 16, 101], f32)
    v.memset(mks[:, :, :], 1.0)
    v.memset(mks[:, :, 0:1], 0.0)
    v.tensor_tensor_scan(
        Dh[:, :, :].rearrange("p a b -> p (a b)"),
        mks[:, :, :].rearrange("p a b -> p (a b)"),
        d2e[:, :, :].rearrange("p a b -> p (a b)"),
        0.0, ALU.mult, ALU.add)
    dctx.__exit__(None, None, None)
    return dict()


# ==== sampling ====


BUCK = 124          # buckets per ray (width 2^-6; bt clamped at 123)
BSP = 16 * BUCK     # 1984
bf16 = mybir.dt.bfloat16

# out = Src0*Src1 - One  (select: keep*(key+1) - 1 -> key if keep else -1)
OP_MUL_SUB1 = lambda: register_op(
    "ANT_EVO_MULSUB1",
    Spec(body=Src0 * Src1 - One,
         reference=lambda in0, in1, s0, s1, imm2: (
             in0.astype(np.float32) * in1 - 1.0)))

# out = (Src0*C0 + C1) + Src1
OP_AFF2 = lambda: register_op(
    "ANT_EVO_AFF2",
    Spec(body=(Src0 * C0 + C1) + Src1,
         reference=lambda in0, in1, s0, s1, imm2: (
             in0.astype(np.float32) * s0 + s1) + in1))


def host_consts():
    """Constant helper tensors shipped from host (tiled to 128 partitions)."""
    j = np.arange(16, dtype=np.int64)
    t = np.arange(101, dtype=np.int64)
    s64 = np.arange(64, dtype=np.int64)
    out = {}
    out["gvals"] = (j[:, None] * 128 + t[None, :] + 1).astype(np.int16).reshape(-1)      # [1616] i16
    out["boffT"] = (j[:, None] * BUCK + 0 * t[None, :]).astype(np.int16).reshape(-1)     # [1616] i16
    out["boffZp1"] = (j[:, None] * BUCK + 1 + 0 * s64[None, :]).astype(np.float32).reshape(-1)  # [1024] f32
    out["sglob1"] = (j[:, None] * 64 + s64[None, :] + 1).astype(np.int16).reshape(-1)    # [1024] i16
    out["toffm"] = (j[:, None] * 102 + 0 * s64[None, :]).astype(np.float32).reshape(-1)  # [1024] f32
    out["soff128"] = (j[:, None] * 128 + 0 * s64[None, :]).astype(np.float32).reshape(-1)  # [1024] f32
    return {k: np.tile(v[None, :], (128, 1)).copy() for k, v in out.items()}


CONST_SPECS = (("gvals", "i16", 1616), ("boffT", "i16", 1616),
               ("boffZp1", "f32", 1024), ("sglob1", "i16", 1024),
               ("toffm", "f32", 1024), ("soff128", "f32", 1024))


def build_sampling(nc, tc, pool, H, Dh, zc, consts_dram, cvec, out_dram):
    """H: [128,101,48] SBUF fp32; Dh: [128,16,101] SBUF fp32; zc: DRAM [128,1024];
    consts_dram: name->DRAM handle; cvec: ior_center floats; out_dram [128,3072]."""
    v = nc.vector
    s = nc.scalar
    g = nc.gpsimd
    sq2 = OP_SQ2()
    sqa = OP_SQA()
    msub1 = OP_MUL_SUB1()
    aff = OP_AFF2()

    # ---- load z and consts (persistent ones in pool; phase consts in p1)
    zt = pool.tile([128, 1024], f32)
    nc.sync.dma_start(zt[:, :], zc[:, :])
    zf = zt[:, :]
    p1ctx = tc.tile_pool(name="smp_p1", bufs=1)
    p1 = p1ctx.__enter__()
    C = {}
    for name, dt_, n in CONST_SPECS:
        pl = pool if name in ("sglob1", "toffm") else p1
        C[name] = pl.tile([128, n], {"i16": i16, "f32": f32}[dt_], name="c_" + name)
        nc.sync.dma_start(C[name][:, :], consts_dram[name][:, :])
    fscr = "f32scr"  # shared-slot tag for sequential f32 scratch [128,1616 max]

    # ---- T-space channels (prep on ACT, overlaps the DVE/Pool LUT build) ----
    Du3 = Dh[:, :, :].rearrange("p a b -> p (a b)").bitcast(u16).rearrange(
        "p (a b h) -> p a b h", b=101, h=2)
    Dhi = pool.tile([128, 16, 102], i16)
    Dlo = pool.tile([128, 16, 102], i16)
    s.activation(Dhi[:, :, 0:101], Du3[:, :, :, 1].bitcast(i16), AF.Copy)
    s.activation(Dlo[:, :, 0:101], Du3[:, :, :, 0].bitcast(i16), AF.Copy)
    Dhi_f = Dhi[:, :, :].rearrange("p a b -> p (a b)")
    Dlo_f = Dlo[:, :, :].rearrange("p a b -> p (a b)")
    Hu = H[:, :, :].rearrange("p a b -> p (a b)").bitcast(u16)
    ychT = {}
    for ci in range(3):
        for half in range(2):
            nm = f"y{ci}h{half}"
            src = Hu.rearrange("p (t j c) -> p j t c", t=101, j=16)[:, :, :, ci * 2 + half]
            tch = pool.tile([128, 16, 102], i16, name="chT_" + nm)
            s.activation(tch[:, :, 0:101], src.bitcast(i16), AF.Copy)
            ychT[nm] = tch

    # ---- S1: bt = clamp(floor(D*64),123) ; posT = bt + ray*124 (i16)
    # exact floor: candidate = round(D*64 - 0.499); fix overshoot (frac>=0.999)
    Dflat = Dh[:, :, :].rearrange("p a b -> p (a b)")
    d64 = p1.tile([128, 1616], f32, tag="dgf")
    v.tensor_scalar_mul(d64[:, :], Dflat, 64.0)          # exact (power of 2)
    btf = p1.tile([128, 1616], f32, tag=fscr)
    v.tensor_scalar_add(btf[:, :], d64[:, :], -0.499)
    bt16 = p1.tile([128, 1616], i16, tag="i16scr")
    v.tensor_scalar_min(bt16[:, :], btf[:, :], 123.0)    # cast: round-nearest
    btf2 = p1.tile([128, 1616], f32, tag=fscr)
    v.tensor_copy(btf2[:, :], bt16[:, :])
    over = p1.tile([128, 1616], f32, tag="i16scr2")
    v.tensor_tensor(over[:, :], btf2[:, :], d64[:, :], ALU.is_gt)
    v.tensor_tensor(bt16[:, :], bt16[:, :], over[:, :], ALU.subtract)
    posT = p1.tile([128, 1616], i16)
    v.tensor_tensor(posT[:, :], bt16[:, :], C["boffT"][:, :], ALU.add)

    # ---- S4: floored bz (as f32) ; posZ1 = bz + ray*124 + 1 (f32)
    bzf = p1.tile([128, 1024], f32, tag=fscr)
    v.tensor_scalar(bzf[:, :], zf, 64.0, scalar2=-0.499, op0=ALU.mult, op1=ALU.add)
    bzi = p1.tile([128, 1024], i16)
    v.tensor_copy(bzi[:, :], bzf[:, :])            # round-nearest = floor(z*64)
    bzff = p1.tile([128, 1024], f32, tag="ubz")
    v.tensor_copy(bzff[:, :], bzi[:, :])           # exact floored value in f32
    posZ1 = p1.tile([128, 1024], f32, tag=fscr)
    v.tensor_tensor(posZ1[:, :], bzff[:, :], C["boffZp1"][:, :], ALU.add)

    # ---- S5/S6: keep-last-of-bucket mask; sigma-scatter U[bucket]=sglob+1
    kpZ = p1.tile([128, 16, 64], f32)
    bz3 = bzff[:, :].rearrange("p (a b) -> p a b", b=64)
    v.tensor_tensor(kpZ[:, :, 0:63], bz3[:, :, 1:64], bz3[:, :, 0:63], ALU.is_gt)
    v.memset(kpZ[:, :, 63:64], 1.0)
    kpZf = kpZ[:, :, :].rearrange("p a b -> p (a b)")
    nkZ = p1.tile([128, 1024], f32)
    s.activation(nkZ[:, :], kpZf, AF.Copy, bias=1.0, scale=-1.0)
    idxZ = p1.tile([128, 1024], i16, tag="i16scr")
    v._custom_dve(msub1, out=idxZ[:, :], in0=kpZf, in1=posZ1[:, :])
    U = p1.tile([128, BSP], i16, tag="ubz")
    g.local_scatter(U[:, :], C["sglob1"][:, :], idxZ[:, :],
                    channels=128, num_elems=BSP, num_idxs=1024)

    # ---- S2/S3: G LUT
    Gar = p1.tile([128, BSP], i16, tag="i16scr2")
    g.local_scatter(Gar[:, :], C["gvals"][:, :], posT[:, :],
                    channels=128, num_elems=BSP, num_idxs=1616)
    Gf = p1.tile([128, BSP], i16, tag="dgf")
    v.tensor_tensor_scan(Gf[:, :], Gar[:, :], Gar[:, :], 0.0, ALU.max, ALU.max)

    # ---- S7: G -> samples (scatter-back by U-1), backward fill, strip
    Um1 = p1.tile([128, BSP], i16, tag="i16scr2")
    v.tensor_scalar_add(Um1[:, :], U[:, :], -1.0)
    cnt0r = p1.tile([128, 1024], i16)
    g.local_scatter(cnt0r[:, :], Gf[:, :], Um1[:, :],
                    channels=128, num_elems=1024, num_idxs=BSP)
    cnt0f = p1.tile([128, 1024], f32, tag=fscr)
    v.tensor_tensor_scan(cnt0f[:, ::-1], nkZ[:, ::-1], cnt0r[:, ::-1],
                         0.0, ALU.mult, ALU.add)
    cnt0 = pool.tile([128, 1024], f32)
    v.tensor_tensor(cnt0[:, :], cnt0f[:, :], C["soff128"][:, :], ALU.subtract)

    slot_pool = [p1]
    # ---- helpers ------------------------------------------------------------
    def build_slot(key_f, kp, nk, SLOT, SLOTp, tag):
        """key_f [128,1024] f32 = (cnt-like) + ray*102; keys nondecr per ray.
        SLOT/SLOTp: [128,1632] i16 tiles."""
        k3 = key_f.rearrange("p (a b) -> p a b", b=64)
        v.tensor_tensor(kp[:, :, 0:63], k3[:, :, 1:64], k3[:, :, 0:63], ALU.is_gt)
        v.memset(kp[:, :, 63:64], 1.0)
        kpf = kp[:, :, :].rearrange("p a b -> p (a b)")
        s.activation(nk[:, :], kpf, AF.Copy, bias=1.0, scale=-1.0)
        idxs = slot_pool[0].tile([128, 1024], i16, name="idxs_" + tag)
        v._custom_dve(msub1, out=idxs[:, :], in0=kpf, in1=key_f)
        g.local_scatter(SLOT[:, :], C["sglob1"][:, :], idxs[:, :],
                        channels=128, num_elems=1632, num_idxs=1024)
        v.tensor_scalar_add(SLOTp[:, :], SLOT[:, :], -1.0)

    def deliver(SLOTp, nk, data_ap, out_t, tag, dt_=i16):
        raw = slot_pool[0].tile([128, 1024], dt_, name="raw_" + tag, tag="rawch")
        g.local_scatter(raw[:, :], data_ap, SLOTp[:, :],
                        channels=128, num_elems=1024, num_idxs=1632)
        v.tensor_tensor_scan(out_t[:, ::-1], nk[:, ::-1], raw[:, ::-1],
                             0.0, ALU.mult, ALU.add)

    def recombine(hi_t, lo_t, out_t):
        loI = slot_pool[0].tile([128, 1024], i32, name="loI", tag="loI")
        v.tensor_copy(out_t[:, :], hi_t[:, :].bitcast(u16))
        v.tensor_scalar(out_t[:, :], out_t[:, :], 16, scalar2=None,
                        op0=ALU.logical_shift_left)
        v.tensor_copy(loI[:, :], lo_t[:, :].bitcast(u16))
        v.tensor_tensor(out_t[:, :], out_t[:, :], loI[:, :], ALU.bitwise_or)


    # ---- correction round: D @ (cnt0-1) -------------------------------------
    p1ctx.__exit__(None, None, None)
    p1bctx = tc.tile_pool(name="smp_p1b", bufs=1)
    p1b = p1bctx.__enter__()
    slot_pool[0] = p1b
    key0 = p1b.tile([128, 1024], f32)
    v.tensor_tensor(key0[:, :], cnt0[:, :], C["toffm"][:, :], ALU.add)
    SLOT = pool.tile([128, 1632], i16)
    SLOTp = pool.tile([128, 1632], i16)
    kp0 = p1b.tile([128, 16, 64], f32, name="kp0")
    nk0 = p1b.tile([128, 1024], f32, name="nk0")
    build_slot(key0[:, :], kp0, nk0, SLOT, SLOTp, "k0")
    dhi0 = p1b.tile([128, 1024], i16, name="dhi0")
    dlo0 = p1b.tile([128, 1024], i16, name="dlo0")
    deliver(SLOTp, nk0, Dhi_f, dhi0, "dh0")
    deliver(SLOTp, nk0, Dlo_f, dlo0, "dl0")
    Dv0 = p1b.tile([128, 1024], i32, name="Dv0")
    recombine(dhi0, dlo0, Dv0)
    corr = p1b.tile([128, 1024], f32)
    v.tensor_tensor(corr[:, :], Dv0[:, :].bitcast(f32), zf, ALU.is_ge)
    cnt = pool.tile([128, 1024], f32)
    v.tensor_tensor(cnt[:, :], cnt0[:, :], corr[:, :], ALU.subtract)
    p1bctx.__exit__(None, None, None)
    p2ctx = tc.tile_pool(name="smp_p2", bufs=1)
    p2 = p2ctx.__enter__()
    slot_pool[0] = p2

    # ---- main delivery keyed idx_pos = cnt-1 --------------------------------
    key1 = p2.tile([128, 1024], f32)
    v.tensor_tensor(key1[:, :], cnt[:, :], C["toffm"][:, :], ALU.add)
    kp1 = pool.tile([128, 16, 64], f32, name="kp1")
    nk1 = pool.tile([128, 1024], f32, name="nk1")
    build_slot(key1[:, :], kp1, nk1, SLOT, SLOTp, "k1")

    ch = {}
    for nm, ap_ in (("dhi", Dhi_f), ("dlo", Dlo_f)):
        t_ = pool.tile([128, 1024], i16, name="ch_" + nm)
        deliver(SLOTp, nk1, ap_, t_, nm)
        ch[nm] = t_
    for ci in range(3):
        for half in range(2):
            nm = f"y{ci}h{half}"
            d_ = pool.tile([128, 1024], i16, name="ch_" + nm)
            deliver(SLOTp, nk1,
                    ychT[nm][:, :, :].rearrange("p a b -> p (a b)"), d_, nm)
            ch[nm] = d_
    H3f = H[:, :, :].rearrange("p a (j c) -> p a j c", c=3)
    for ci in range(3):
        nm = f"d{ci}"
        tch = p2.tile([128, 16, 102], bf16, name="chT_" + nm, tag="chTd")
        v.memset(tch[:, :, 100:102], 0.0)
        v.tensor_tensor(tch[:, :, 0:100].rearrange("p a b -> p b a"),
                        H3f[:, 1:101, :, ci], H3f[:, 0:100, :, ci], ALU.subtract)
        d_ = pool.tile([128, 1024], bf16, name="ch_" + nm)
        deliver(SLOTp, nk1, tch[:, :, :].rearrange("p a b -> p (a b)"), d_, nm,
                dt_=bf16)
        ch[nm] = d_

    Dpos = pool.tile([128, 1024], i32, name="Dpos")
    recombine(ch["dhi"], ch["dlo"], Dpos)
    y0 = []
    for ci in range(3):
        t_ = pool.tile([128, 1024], i32, name=f"y0_{ci}")
        recombine(ch[f"y{ci}h1"], ch[f"y{ci}h0"], t_)
        y0.append(t_[:, :].bitcast(f32))

    # ---- final math ----------------------------------------------------------
    wrap = p2.tile([128, 1024], i16)
    v.tensor_scalar(wrap[:, :], cnt[:, :], 100.5, scalar2=None, op0=ALU.is_gt)
    dl = []
    for ci in range(3):
        dfull = pool.tile([128, 1024], f32, name=f"df_{ci}")
        v.tensor_copy(dfull[:, :], ch[f"d{ci}"][:, :])
        patch = p2.tile([128, 1024], f32, name=f"pt_{ci}", tag="patch")
        yib = H3f[:, 0, :, ci].rearrange("p (a o) -> p a o", o=1).to_broadcast(
            [128, 16, 64])
        v.tensor_tensor(patch[:, :].rearrange("p (a b) -> p a b", b=64), yib,
                        y0[ci].rearrange("p (a b) -> p a b", b=64), ALU.subtract)
        v.copy_predicated(dfull[:, :], wrap[:, :], patch[:, :])
        dl.append(dfull)
    msq = p2.tile([128, 1024], f32)
    v._custom_dve(sq2, out=msq[:, :], in0=dl[0][:, :], in1=dl[1][:, :])
    v._custom_dve(sqa, out=msq[:, :], in0=dl[2][:, :], in1=msq[:, :])
    inv = p2.tile([128, 1024], f32)
    scr = p2.tile([128, 1024], f32, name="scr_inv")
    v.reciprocal_approx_accurate(inv[:, :], msq[:, :], scr[:, :])
    rn = p2.tile([128, 1024], f32)
    s.activation(rn[:, :], inv[:, :], AF.Sqrt)
    sc = pool.tile([128, 1024], f32)
    v.tensor_tensor(sc[:, :], zf, Dpos[:, :].bitcast(f32), ALU.subtract)
    v.tensor_tensor(sc[:, :], sc[:, :], rn[:, :], ALU.mult)
    out3 = pool.tile([128, 3072], f32)
    o3 = out3[:, :].rearrange("p (s c) -> p s c", c=3)
    for ci in range(3):
        t_ = p2.tile([128, 1024], f32, name=f"sm_{ci}", tag="sm")
        v.tensor_tensor(t_[:, :], sc[:, :], dl[ci][:, :], ALU.mult)
        v._custom_dve(aff, out=o3[:, :, ci], in0=t_[:, :], in1=y0[ci],
                      s0=1.0, s1=float(np.float32(cvec[ci])))
    nc.sync.dma_start(out_dram[:, :], out3[:, :])
    p2ctx.__exit__(None, None, None)
    return dict()


# ---------------------------------------------------------------------------
_BUILD_CACHE = {}


def _build(A, cvec, n_cores=8):
    key = (float(np.float32(A)), tuple(float(np.float32(x)) for x in cvec))
    if key in _BUILD_CACHE:
        return _BUILD_CACHE[key]
    nc = bacc.Bacc("TRN2", target_bir_lowering=False, debug=False,
                   num_devices=n_cores)
    x0c = nc.dram_tensor("x0c", [128, 48], f32, kind="ExternalInput")
    v0c = nc.dram_tensor("v0c", [128, 48], f32, kind="ExternalInput")
    zc = nc.dram_tensor("zc", [128, 1024], f32, kind="ExternalInput")
    cdr = {}
    for name, dt_, n in CONST_SPECS:
        cdr[name] = nc.dram_tensor("cst_" + name, [128, n],
                                   {"i16": i16, "f32": f32}[dt_],
                                   kind="ExternalInput")
    Oout = nc.dram_tensor("Oout", [128, 3072], f32, kind="ExternalOutput")
    with TileContext(nc) as tc:
        with tc.tile_pool(name="pp", bufs=1) as pool:
            H = pool.tile([128, 101, 48], f32)
            Dh = pool.tile([128, 16, 101], f32)
            build_integration(nc, tc, pool, x0c, v0c, A, cvec, H, Dh)
            build_sampling(nc, tc, pool, H, Dh, zc, cdr, cvec, Oout)
    nc.compile()
    _BUILD_CACHE[key] = nc
    return nc


def kernel(x0, v0, z_vals, ior_center, ior_amp):
    """Full inputs -> full output [16384, 64, 3] float32."""
    x0 = np.ascontiguousarray(np.asarray(x0, np.float32))
    v0 = np.ascontiguousarray(np.asarray(v0, np.float32))
    z = np.ascontiguousarray(np.asarray(z_vals, np.float32)).reshape(16384, 64)
    c = np.asarray(ior_center, np.float32).reshape(3)
    A = float(np.asarray(ior_amp, np.float32).reshape(1)[0])
    n_cores = 8
    nc = _build(A, [float(c[0]), float(c[1]), float(c[2])], n_cores)
    cst = host_consts()
    in_maps = []
    for core in range(n_cores):
        sl = slice(core * 2048, (core + 1) * 2048)
        m = {"x0c": x0[sl].reshape(128, 48).copy(),
             "v0c": v0[sl].reshape(128, 48).copy(),
             "zc": z[sl].reshape(128, 1024).copy()}
        m.update({"cst_" + k: v for k, v in cst.items()})
        in_maps.append(m)
    res = run_bass_kernel_spmd(nc, in_maps, core_ids=list(range(n_cores)))
    out = np.empty((16384, 64, 3), np.float32)
    for core in range(n_cores):
        sl = slice(core * 2048, (core + 1) * 2048)
        out[sl] = res.results[core]["Oout"].reshape(2048, 64, 3)
    return out



# revision 8
# speedup vs baseline: 1.4624x; 1.4624x over previous
"""TRN2 Bass kernel for nn_EvolutionModel_91173565759692 (self-contained).

Physics: 16384 rays, 100-step velocity-Verlet in ior-center-centered coords
  y_{t+1} = W(g)*y_t - y_{t-1},  W = (g*c2 + c1)*g + 2,  g = exp(-2|y|^2)
exp computed as g ~= (1 - alpha*r2)^128 via repeated squaring, which fuses
the whole step into 4 DVE ops (SQ2S, V16, QW, sub).
Sampling: bucket LUT (width 2^-6) searchsorted, fp16 payload channels, no
correction round (off-by-one brackets are collinear => error ~1e-3 << tol).
8-way data-parallel over rays (2048 rays/core).
"""
import sys
sys.path.insert(0, "/opt/trn_rl_repo")
import numpy as np
import concourse.bass as bass
import concourse.bacc as bacc
import concourse.mybir as mybir
from concourse.tile import TileContext
import concourse.dve_ops as dve_ops
from concourse import dve_spec
from concourse.dve_spec import Spec, Src0, Src1, C0, C1, C2, One, sq, lower
from concourse.dve_uop import DveOpSpec
from concourse.dve_table_gen import dve_ver_for
from concourse.bass_utils import run_bass_kernel_spmd

f32 = mybir.dt.float32
f16 = mybir.dt.float16
i16 = mybir.dt.int16
AF = mybir.ActivationFunctionType
ALU = mybir.AluOpType

N_STEPS = 100
DT = np.float32(0.02)
KC = np.float32(-DT * DT / np.float32(0.25))   # -dt^2/sigma^2
ALPHA = float(np.float32((2.0 / 128.0) * (1.0 - 1.0 / 128.0)))

_registered = {}


def register_op(name, spec, subdim=False):
    if name in _registered:
        return _registered[name]
    ver = dve_ver_for("TRN2")
    row = dve_ops._CUSTOM_DVE_ROW_BASE + len(dve_ops.OPS)
    assert row < 0x20
    dve_ops._SUB_OPCODE_FOR_NAME[name] = row
    tmp = DveOpSpec(name=name, opcode=row, uops=lower(spec, ver=ver),
                    rd1_en=dve_spec._has_src1(spec))
    op = dve_ops.DveOp(name, spec, subdim, {ver: tmp.sha(ver)})
    dve_ops.OPS.append(op)
    dve_ops.CUSTOM_DVE_SPECS[name] = spec
    _registered[name] = op
    return op


# t12 = (y0^2 + y1^2) * alpha
OP_SQ2S = lambda: register_op(
    "ANT2_SQ2S",
    Spec(body=(sq(Src0) + sq(Src1)) * C0,
         reference=lambda in0, in1, s0, s1, imm2: (
             (in0.astype(np.float32) ** 2 + in1.astype(np.float32) ** 2) * s0)))


# v16 = (1 - (alpha*y2^2 + t12))^16
def _v16_ref(in0, in1, s0, s1, imm2):
    v = 1.0 - (in0.astype(np.float32) ** 2 * s0 + in1.astype(np.float32))
    for _ in range(4):
        v = v * v
    return v


def _v16_body():
    v = One - (sq(Src0) * C0 + Src1)
    for _ in range(4):
        v = sq(v)
    return v


OP_V16 = lambda: register_op("ANT2_V16", Spec(body=_v16_body(), reference=_v16_ref))


# q = y * ((g*c2 + c1)*g + k), g = Src1^8 (Src1 = v16 bcast), k = 2 or 1
def _qwk_body(k):
    g = sq(sq(sq(Src1)))
    w = (g * C0 + C1) * g + (One + One if k == 2 else One)
    return Src0 * w


def _qwk_ref(k):
    def ref(in0, in1, s0, s1, imm2):
        g = in1.astype(np.float32)
        for _ in range(3):
            g = g * g
        return in0.astype(np.float32) * ((g * s0 + s1) * g + float(k))
    return ref


OP_QW2N = lambda: register_op("ANT2_QW2", Spec(body=_qwk_body(2), reference=_qwk_ref(2)))
OP_QW1N = lambda: register_op("ANT2_QW1", Spec(body=_qwk_body(1), reference=_qwk_ref(1)))

# out = Src0*Src0 + Src1*Src1
OP_SQ2 = lambda: register_op(
    "ANT_EVO_SQ2",
    Spec(body=Src0 * Src0 + Src1 * Src1,
         reference=lambda in0, in1, s0, s1, imm2: (
             in0.astype(np.float32) ** 2 + in1.astype(np.float32) ** 2)))

# out = Src0*Src0 + Src1
OP_SQA = lambda: register_op(
    "ANT_EVO_SQA",
    Spec(body=Src0 * Src0 + Src1,
         reference=lambda in0, in1, s0, s1, imm2: (
             in0.astype(np.float32) ** 2 + in1.astype(np.float32))))

# out = Src0*Src1 - One
OP_MUL_SUB1 = lambda: register_op(
    "ANT_EVO_MULSUB1",
    Spec(body=Src0 * Src1 - One,
         reference=lambda in0, in1, s0, s1, imm2: (
             in0.astype(np.float32) * in1 - 1.0)))

# out = (Src0*C0 + C1) + Src1
OP_AFF2 = lambda: register_op(
    "ANT_EVO_AFF2",
    Spec(body=(Src0 * C0 + C1) + Src1,
         reference=lambda in0, in1, s0, s1, imm2: (
             in0.astype(np.float32) * s0 + s1) + in1))


BUCK = 124          # buckets per ray (width 2^-6; bt clamped at 123)
BSP = 16 * BUCK     # 1984
TS = 102            # T-slots per ray (101 steps + pad)
NTS = 16 * TS       # 1632

CONST_SPECS = (("gvals", "i16", NTS), ("cboffT", "f32", NTS),
               ("cboffZ1", "f16", 1024), ("sglob1", "i16", 1024),
               ("cfold2", "f32", 1024), ("cwrap", "f32", 1024))
_DTMAP = {"i16": i16, "f32": f32, "f16": f16}


def host_consts():
    """Constant helper tensors (tiled to 128 partitions)."""
    j = np.arange(16, dtype=np.int64)[:, None]
    t = np.arange(TS, dtype=np.int64)[None, :]
    s64 = np.arange(64, dtype=np.int64)[None, :]
    out = {}
    gv = (j * 128 + t + 1).astype(np.int16)
    gv[:, 101] = 0
    out["gvals"] = gv.reshape(-1)
    cb = (j * BUCK - 0.499 + 0 * t).astype(np.float32)
    cb[:, 101] = -10000.0
    out["cboffT"] = cb.reshape(-1)
    out["cboffZ1"] = (j * BUCK + 1.0 + 0 * s64).astype(np.float16).reshape(-1)
    out["sglob1"] = (j * 64 + s64 + 1).astype(np.int16).reshape(-1)
    out["cfold2"] = (j * TS - j * 128 + 0 * s64).astype(np.float32).reshape(-1)
    out["cwrap"] = (j * TS + 100.5 + 0 * s64).astype(np.float32).reshape(-1)
    return {k: np.tile(v[None, :], (128, 1)).copy() for k, v in out.items()}


def build_integration(nc, tc, pool, x0c, v0c, A, cvec, H):
    """100-step loop -> H [128,101,48] f32 SBUF."""
    v = nc.vector
    sq2s = OP_SQ2S()
    v16op = OP_V16()
    qw2 = OP_QW2N()
    qw1 = OP_QW1N()

    A = float(np.float32(A))
    c1f = float(np.float32(KC) * np.float32(A))
    c2f = float(np.float32(c1f) * np.float32(A))
    c1hf = float(np.float32(c1f) * np.float32(0.5))
    c2hf = float(np.float32(c2f) * np.float32(0.5))

    x0t = pool.tile([128, 48], f32)
    nc.sync.dma_start(x0t[:, :], x0c[:, :])
    u0 = pool.tile([128, 48], f32)
    nc.sync.dma_start(u0[:, :], v0c[:, :])
    v.tensor_scalar_mul(u0[:, :], u0[:, :], float(DT))  # u0 = dt*v0

    H3 = H  # [128, 101, 48]
    x03 = x0t[:, :].rearrange("p (a c) -> p a c", c=3)
    h03 = H3[:, 0, :].rearrange("p (a c) -> p a c", c=3)
    for ci in range(3):
        v.tensor_scalar_add(h03[:, :, ci], x03[:, :, ci], -float(np.float32(cvec[ci])))

    t12 = pool.tile([128, 16], f32, name="t12")
    v16t = pool.tile([128, 16], f32, name="v16t")
    q = pool.tile([128, 48], f32, name="qtile")

    def yv(t):  # [128, 16, 3] view of hist at step t
        return H3[:, t, :].rearrange("p (a c) -> p a c", c=3)

    def step(t, op, c1x, c2x):
        y3 = yv(t)
        v._custom_dve(sq2s, out=t12[:, :], in0=y3[:, :, 0], in1=y3[:, :, 1],
                      s0=ALPHA)
        v._custom_dve(v16op, out=v16t[:, :], in0=y3[:, :, 2], in1=t12[:, :],
                      s0=ALPHA)
        gb = v16t[:, :].rearrange("p (a o) -> p a o", o=1).to_broadcast(
            [128, 16, 3])
        v._custom_dve(op, out=q[:, :].rearrange("p (a c) -> p a c", c=3),
                      in0=y3, in1=gb, s0=c2x, s1=c1x)

    step(0, qw1, c1hf, c2hf)                         # y1 = W1*y0 + dt*v0
    v.tensor_tensor(H3[:, 1, :], q[:, :], u0[:, :], ALU.add)
    for t in range(1, N_STEPS):                      # y_{t+1} = W*y_t - y_{t-1}
        step(t, qw2, c1f, c2f)
        v.tensor_tensor(H3[:, t + 1, :], q[:, :], H3[:, t - 1, :], ALU.subtract)


def build_sampling(nc, tc, pool, H, zc, consts_dram, cvec, out_dram):
    """H: [128,101,48] SBUF f32; zc: DRAM [128,1024]; out_dram [128,3072]."""
    v = nc.vector
    s = nc.scalar
    g = nc.gpsimd
    sq2 = OP_SQ2()
    sqa = OP_SQA()
    msub1 = OP_MUL_SUB1()
    aff = OP_AFF2()

    zt = pool.tile([128, 1024], f32)
    nc.sync.dma_start(zt[:, :], zc[:, :])
    C = {}
    for name, dt_, n in CONST_SPECS:
        C[name] = pool.tile([128, n], _DTMAP[dt_], name="c_" + name)
        nc.sync.dma_start(C[name][:, :], consts_dram[name][:, :])

    # ====== z-side chain: all on Pool/ACT, overlaps the integration loop ====
    nkZ = pool.tile([128, 1024], f32, name="nkZ")
    U = pool.tile([128, BSP], i16, name="U")
    zctx = tc.tile_pool(name="zscr", bufs=1)
    zp = zctx.__enter__()
    bzr = zp.tile([128, 1024], i16, name="bzr")
    v.tensor_scalar(bzr[:, :], zt[:, :], 64.0, scalar2=-0.499,
                    op0=ALU.mult, op1=ALU.add)          # round -> floor(z*64)
    bzh = zp.tile([128, 1024], f16, name="bzh")
    v.tensor_copy(bzh[:, :], bzr[:, :])
    posZ1 = zp.tile([128, 1024], f16, name="posZ1")
    v.tensor_tensor(posZ1[:, :], bzh[:, :], C["cboffZ1"][:, :], ALU.add)
    kpZ = zp.tile([128, 16, 64], f32, name="kpZ")
    bz3 = bzr[:, :].rearrange("p (a b) -> p a b", b=64)
    v.tensor_tensor(kpZ[:, :, 0:63], bz3[:, :, 1:64], bz3[:, :, 0:63], ALU.is_gt)
    v.memset(kpZ[:, :, 63:64], 1.0)
    kpZf = kpZ[:, :, :].rearrange("p a b -> p (a b)")
    s.activation(nkZ[:, :], kpZf, AF.Copy, bias=1.0, scale=-1.0)
    idxZ = zp.tile([128, 1024], i16, name="idxZ")
    v._custom_dve(msub1, out=idxZ[:, :], in0=kpZf, in1=posZ1[:, :])
    g.local_scatter(U[:, :], C["sglob1"][:, :], idxZ[:, :],
                    channels=128, num_elems=BSP, num_idxs=1024)
    v.tensor_scalar_add(U[:, :], U[:, :], -1.0)          # U-1 (in-place)
    zctx.__exit__(None, None, None)

    # ================= T-side prep (after integration) ======================
    H3f = H[:, :, :].rearrange("p a (j c) -> p a j c", c=3)
    lctx = tc.tile_pool(name="lutscr", bufs=1)
    lp = lctx.__enter__()
    dch = []
    for ci in range(3):
        t_ = pool.tile([128, 16, TS], f16, name=f"dch{ci}")
        v.memset(t_[:, :, 100:102], 0.0)
        v.tensor_tensor(t_[:, :, 0:100].rearrange("p a b -> p b a"),
                        H3f[:, 1:101, :, ci], H3f[:, 0:100, :, ci], ALU.subtract)
        v.tensor_copy(t_[:, :, 100:101], t_[:, :, 99:100])   # dup-last-delta
        dch.append(t_)
    t2 = lp.tile([128, 16, 100], f32, name="t2scr")
    v._custom_dve(sq2, out=t2[:, :, :],
                  in0=dch[0][:, :, 0:100], in1=dch[1][:, :, 0:100])
    d2e = pool.tile([128, 16, TS], f32, name="d2e")
    v.memset(d2e[:, :, 0:1], 0.0)
    v.memset(d2e[:, :, 101:102], 0.0)
    v._custom_dve(sqa, out=d2e[:, :, 1:101],
                  in0=dch[2][:, :, 0:100], in1=t2[:, :, :])
    s.activation(d2e[:, :, 1:101], d2e[:, :, 1:101], AF.Sqrt)
    mks = lp.tile([128, 16, TS], f32, name="mks")
    v.memset(mks[:, :, :], 1.0)
    v.memset(mks[:, :, 0:1], 0.0)
    d2f = d2e[:, :, :].rearrange("p a b -> p (a b)")
    v.tensor_tensor_scan(d2f, mks[:, :, :].rearrange("p a b -> p (a b)"),
                         d2f, 0.0, ALU.mult, ALU.add)    # in-place cumsum -> D
    Dflat = d2f
    # fp16 payload channels: D and y (on ACT, off critical path)
    Dch = pool.tile([128, 16, TS], f16, name="Dch")
    s.activation(Dch[:, :, :].rearrange("p a b -> p (a b)"), Dflat, AF.Copy)
    ych = []
    for ci in range(3):
        t_ = pool.tile([128, 16, TS], f16, name=f"ych{ci}")
        v.memset(t_[:, :, 101:102], 0.0)
        src = H[:, :, :].rearrange("p a (j c) -> p j a c", c=3)[:, :, :, ci]
        s.activation(t_[:, :, 0:101], src, AF.Copy)
        ych.append(t_)

    # ================= bucket LUT -> cnt0 -> key1p ==========================
    btm = lp.tile([128, NTS], f32, name="btm")
    v.tensor_scalar(btm[:, :], Dflat, 64.0, scalar2=123.3, op0=ALU.mult, op1=ALU.min)
    posT = lp.tile([128, NTS], i16, name="posT")
    v.tensor_tensor(posT[:, :], btm[:, :], C["cboffT"][:, :], ALU.add)
    Gar = lp.tile([128, BSP], i16, name="Gar")
    g.local_scatter(Gar[:, :], C["gvals"][:, :], posT[:, :],
                    channels=128, num_elems=BSP, num_idxs=NTS)
    Gf = lp.tile([128, BSP], i16, name="Gf")
    v.tensor_tensor_scan(Gf[:, :], Gar[:, :], Gar[:, :], 0.0, ALU.max, ALU.max)
    cnt0r = lp.tile([128, 1024], i16, name="cnt0r")
    g.local_scatter(cnt0r[:, :], Gf[:, :], U[:, :],
                    channels=128, num_elems=1024, num_idxs=BSP)
    key1p = pool.tile([128, 1024], f32, name="key1p")
    v.tensor_tensor_scan(key1p[:, ::-1], nkZ[:, ::-1], cnt0r[:, ::-1],
                         0.0, ALU.mult, ALU.add)         # backward fill = cnt0
    v.tensor_tensor(key1p[:, :], key1p[:, :], C["cfold2"][:, :], ALU.add)
    lctx.__exit__(None, None, None)

    # ================= SLOT build ===========================================
    kp1 = pool.tile([128, 16, 64], f32, name="kp1")
    k3 = key1p[:, :].rearrange("p (a b) -> p a b", b=64)
    v.tensor_tensor(kp1[:, :, 0:63], k3[:, :, 1:64], k3[:, :, 0:63], ALU.is_gt)
    v.memset(kp1[:, :, 63:64], 1.0)
    kp1f = kp1[:, :, :].rearrange("p a b -> p (a b)")
    nk1 = pool.tile([128, 1024], f32, name="nk1")
    s.activation(nk1[:, :], kp1f, AF.Copy, bias=1.0, scale=-1.0)
    idxs = pool.tile([128, 1024], i16, name="idxs")
    v._custom_dve(msub1, out=idxs[:, :], in0=kp1f, in1=key1p[:, :])
    SLOT = pool.tile([128, NTS], i16, name="SLOT")
    g.local_scatter(SLOT[:, :], C["sglob1"][:, :], idxs[:, :],
                    channels=128, num_elems=NTS, num_idxs=1024)
    v.tensor_scalar_add(SLOT[:, :], SLOT[:, :], -1.0)    # in-place: SLOT-1

    # ================= payload delivery (7 fp16 channels) ===================
    dctx = tc.tile_pool(name="dscr", bufs=1)
    dp = dctx.__enter__()
    rawtags = ["rawA", "rawB", "rawC"]

    def deliver(data_ap, name, k):
        raw = dp.tile([128, 1024], f16, name="raw_" + name, tag=rawtags[k % 3])
        g.local_scatter(raw[:, :], data_ap, SLOT[:, :],
                        channels=128, num_elems=1024, num_idxs=NTS)
        out_t = pool.tile([128, 1024], f16, name="smp_" + name)
        v.tensor_tensor_scan(out_t[:, ::-1], nk1[:, ::-1], raw[:, ::-1],
                             0.0, ALU.mult, ALU.add)
        return out_t

    Dsmp = deliver(Dch[:, :, :].rearrange("p a b -> p (a b)"), "D", 0)
    dsmp = [deliver(dch[ci][:, :, :].rearrange("p a b -> p (a b)"), f"d{ci}", 1 + ci)
            for ci in range(3)]
    ysmp = [deliver(ych[ci][:, :, :].rearrange("p a b -> p (a b)"), f"y{ci}", 4 + ci)
            for ci in range(3)]

    # ================= final math ===========================================
    val = pool.tile([128, 1024], f32, name="val")
    v.tensor_tensor(val[:, :], zt[:, :], Dsmp[:, :], ALU.subtract)
    geo = pool.tile([128, 1024], f32, name="geo")
    v.tensor_scalar(geo[:, :], val[:, :], 0.0, scalar2=None, op0=ALU.is_ge)
    v.tensor_tensor(key1p[:, :], key1p[:, :], C["cwrap"][:, :], ALU.is_gt)
    wrapm = pool.tile([128, 1024], i16, name="wrapm")
    v.tensor_tensor(wrapm[:, :], key1p[:, :], geo[:, :], ALU.mult)
    for ci in range(3):
        pt = dp.tile([128, 1024], f16, name=f"pt{ci}", tag="pt")
        yib = ych[ci][:, :, 0:1].to_broadcast([128, 16, 64])
        v.tensor_tensor(pt[:, :].rearrange("p (a b) -> p a b", b=64), yib,
                        ysmp[ci][:, :].rearrange("p (a b) -> p a b", b=64),
                        ALU.subtract)
        v.copy_predicated(dsmp[ci][:, :], wrapm[:, :], pt[:, :])
    msq = pool.tile([128, 1024], f32, name="msq")
    v._custom_dve(sq2, out=msq[:, :], in0=dsmp[0][:, :], in1=dsmp[1][:, :])
    v._custom_dve(sqa, out=msq[:, :], in0=dsmp[2][:, :], in1=msq[:, :])
    inv = pool.tile([128, 1024], f32, name="inv")
    scr = pool.tile([128, 1024], f32, name="scr_inv")
    v.reciprocal_approx_accurate(inv[:, :], msq[:, :], scr[:, :])
    s.activation(inv[:, :], inv[:, :], AF.Sqrt)          # in-place rsqrt
    v.tensor_tensor(val[:, :], val[:, :], inv[:, :], ALU.mult)  # sc in-place
    out3 = pool.tile([128, 3072], f32, name="out3")
    o3 = out3[:, :].rearrange("p (s c) -> p s c", c=3)
    for ci in range(3):
        t_ = dp.tile([128, 1024], f32, name=f"sm{ci}", tag="sm")
        v.tensor_tensor(t_[:, :], val[:, :], dsmp[ci][:, :], ALU.mult)
        v._custom_dve(aff, out=o3[:, :, ci], in0=t_[:, :], in1=ysmp[ci][:, :],
                      s0=1.0, s1=float(np.float32(cvec[ci])))
    dctx.__exit__(None, None, None)
    nc.sync.dma_start(out_dram[:, :], out3[:, :])


# ---------------------------------------------------------------------------
_BUILD_CACHE = {}


def _build(A, cvec, n_cores=8):
    key = (float(np.float32(A)), tuple(float(np.float32(x)) for x in cvec))
    if key in _BUILD_CACHE:
        return _BUILD_CACHE[key]
    nc = bacc.Bacc("TRN2", target_bir_lowering=False, debug=False,
                   num_devices=n_cores)
    x0c = nc.dram_tensor("x0c", [128, 48], f32, kind="ExternalInput")
    v0c = nc.dram_tensor("v0c", [128, 48], f32, kind="ExternalInput")
    zc = nc.dram_tensor("zc", [128, 1024], f32, kind="ExternalInput")
    cdr = {}
    for name, dt_, n in CONST_SPECS:
        cdr[name] = nc.dram_tensor("cst_" + name, [128, n], _DTMAP[dt_],
                                   kind="ExternalInput")
    Oout = nc.dram_tensor("Oout", [128, 3072], f32, kind="ExternalOutput")
    with TileContext(nc) as tc:
        with tc.tile_pool(name="pp", bufs=1) as pool:
            H = pool.tile([128, 101, 48], f32)
            build_sampling_pre = None
            build_integration(nc, tc, pool, x0c, v0c, A, cvec, H)
            build_sampling(nc, tc, pool, H, zc, cdr, cvec, Oout)
    nc.compile()
    _BUILD_CACHE[key] = nc
    return nc


def kernel(x0, v0, z_vals, ior_center, ior_amp):
    """Full inputs -> full output [16384, 64, 3] float32."""
    x0 = np.ascontiguousarray(np.asarray(x0, np.float32))
    v0 = np.ascontiguousarray(np.asarray(v0, np.float32))
    z = np.ascontiguousarray(np.asarray(z_vals, np.float32)).reshape(16384, 64)
    c = np.asarray(ior_center, np.float32).reshape(3)
    A = float(np.asarray(ior_amp, np.float32).reshape(1)[0])
    n_cores = 8
    nc = _build(A, [float(c[0]), float(c[1]), float(c[2])], n_cores)
    cst = host_consts()
    in_maps = []
    for core in range(n_cores):
        sl = slice(core * 2048, (core + 1) * 2048)
        m = {"x0c": x0[sl].reshape(128, 48).copy(),
             "v0c": v0[sl].reshape(128, 48).copy(),
             "zc": z[sl].reshape(128, 1024).copy()}
        m.update({"cst_" + k: v for k, v in cst.items()})
        in_maps.append(m)
    res = run_bass_kernel_spmd(nc, in_maps, core_ids=list(range(n_cores)))
    out = np.empty((16384, 64, 3), np.float32)
    for core in range(n_cores):
        sl = slice(core * 2048, (core + 1) * 2048)
        out[sl] = res.results[core]["Oout"].reshape(2048, 64, 3)
    return out


# revision 9
# speedup vs baseline: 1.7689x; 1.2096x over previous
"""TRN2 Bass kernel for nn_EvolutionModel_91173565759692 (self-contained).

Physics: 16384 rays, 100-step velocity-Verlet in ior-center-centered coords
  y_{t+1} = W(g)*y_t - y_{t-1},  W = (g*c2 + c1)*g + 2,  g = exp(-2|y|^2)
exp computed as g ~= (1 - alpha*r2)^128 via repeated squaring, which fuses
the whole step into 4 DVE ops (SQ2S, V16, QW, sub).
Sampling: bucket LUT (width 2^-6) searchsorted, fp16 payload channels, no
correction round (off-by-one brackets are collinear => error ~1e-3 << tol).
8-way data-parallel over rays (2048 rays/core).
"""
import sys
sys.path.insert(0, "/opt/trn_rl_repo")
import numpy as np
import concourse.bass as bass
import concourse.bacc as bacc
import concourse.mybir as mybir
from concourse.tile import TileContext
import concourse.dve_ops as dve_ops
from concourse import dve_spec
from concourse.dve_spec import Spec, Src0, Src1, C0, C1, C2, One, sq, lower
from concourse.dve_uop import DveOpSpec
from concourse.dve_table_gen import dve_ver_for
from concourse.bass_utils import run_bass_kernel_spmd

f32 = mybir.dt.float32
f16 = mybir.dt.float16
i16 = mybir.dt.int16
AF = mybir.ActivationFunctionType
ALU = mybir.AluOpType

N_STEPS = 100
DT = np.float32(0.02)
KC = np.float32(-DT * DT / np.float32(0.25))   # -dt^2/sigma^2
ALPHA = float(np.float32((2.0 / 128.0) * (1.0 - 1.0 / 128.0)))

_registered = {}


def register_op(name, spec, subdim=False):
    if name in _registered:
        return _registered[name]
    ver = dve_ver_for("TRN2")
    row = dve_ops._CUSTOM_DVE_ROW_BASE + len(dve_ops.OPS)
    assert row < 0x20
    dve_ops._SUB_OPCODE_FOR_NAME[name] = row
    tmp = DveOpSpec(name=name, opcode=row, uops=lower(spec, ver=ver),
                    rd1_en=dve_spec._has_src1(spec))
    op = dve_ops.DveOp(name, spec, subdim, {ver: tmp.sha(ver)})
    dve_ops.OPS.append(op)
    dve_ops.CUSTOM_DVE_SPECS[name] = spec
    _registered[name] = op
    return op


# t12 = (y0^2 + y1^2) * alpha
OP_SQ2S = lambda: register_op(
    "ANT2_SQ2S",
    Spec(body=(sq(Src0) + sq(Src1)) * C0,
         reference=lambda in0, in1, s0, s1, imm2: (
             (in0.astype(np.float32) ** 2 + in1.astype(np.float32) ** 2) * s0)))


# v16 = (1 - (alpha*y2^2 + t12))^16
def _v16_ref(in0, in1, s0, s1, imm2):
    v = 1.0 - (in0.astype(np.float32) ** 2 * s0 + in1.astype(np.float32))
    for _ in range(4):
        v = v * v
    return v


def _v16_body():
    v = One - (sq(Src0) * C0 + Src1)
    for _ in range(4):
        v = sq(v)
    return v


OP_V16 = lambda: register_op("ANT2_V16", Spec(body=_v16_body(), reference=_v16_ref))


# q = y * ((g*c2 + c1)*g + k), g = Src1^8 (Src1 = v16 bcast), k = 2 or 1
def _qwk_body(k):
    g = sq(sq(sq(Src1)))
    w = (g * C0 + C1) * g + (One + One if k == 2 else One)
    return Src0 * w


def _qwk_ref(k):
    def ref(in0, in1, s0, s1, imm2):
        g = in1.astype(np.float32)
        for _ in range(3):
            g = g * g
        return in0.astype(np.float32) * ((g * s0 + s1) * g + float(k))
    return ref


OP_QW2N = lambda: register_op("ANT2_QW2", Spec(body=_qwk_body(2), reference=_qwk_ref(2)))
OP_QW1N = lambda: register_op("ANT2_QW1", Spec(body=_qwk_body(1), reference=_qwk_ref(1)))

# out = Src0*Src0 + Src1*Src1
OP_SQ2 = lambda: register_op(
    "ANT_EVO_SQ2",
    Spec(body=Src0 * Src0 + Src1 * Src1,
         reference=lambda in0, in1, s0, s1, imm2: (
             in0.astype(np.float32) ** 2 + in1.astype(np.float32) ** 2)))

# out = Src0*Src0 + Src1
OP_SQA = lambda: register_op(
    "ANT_EVO_SQA",
    Spec(body=Src0 * Src0 + Src1,
         reference=lambda in0, in1, s0, s1, imm2: (
             in0.astype(np.float32) ** 2 + in1.astype(np.float32))))

# out = Src0*Src1 - One
OP_MUL_SUB1 = lambda: register_op(
    "ANT_EVO_MULSUB1",
    Spec(body=Src0 * Src1 - One,
         reference=lambda in0, in1, s0, s1, imm2: (
             in0.astype(np.float32) * in1 - 1.0)))

# out = (Src0*C0 + C1) + Src1
OP_AFF2 = lambda: register_op(
    "ANT_EVO_AFF2",
    Spec(body=(Src0 * C0 + C1) + Src1,
         reference=lambda in0, in1, s0, s1, imm2: (
             in0.astype(np.float32) * s0 + s1) + in1))


BUCK = 124          # buckets per ray (width 2^-6; bt clamped at 123)
BSP = 16 * BUCK     # 1984
TS = 102            # T-slots per ray (101 steps + pad)
NTS = 16 * TS       # 1632

CONST_SPECS = (("gvals", "i16", NTS), ("cboffT", "f32", NTS),
               ("cboffZ1", "f16", 1024), ("sglob1", "i16", 1024),
               ("cfold2", "f32", 1024), ("cwrap", "f32", 1024))
_DTMAP = {"i16": i16, "f32": f32, "f16": f16}


def host_consts():
    """Constant helper tensors (tiled to 128 partitions)."""
    j = np.arange(16, dtype=np.int64)[:, None]
    t = np.arange(TS, dtype=np.int64)[None, :]
    s64 = np.arange(64, dtype=np.int64)[None, :]
    out = {}
    gv = (j * 128 + t + 1).astype(np.int16)
    gv[:, 101] = 0
    out["gvals"] = gv.reshape(-1)
    cb = (j * BUCK - 0.499 + 0 * t).astype(np.float32)
    cb[:, 101] = -10000.0
    out["cboffT"] = cb.reshape(-1)
    out["cboffZ1"] = (j * BUCK + 1.0 + 0 * s64).astype(np.float16).reshape(-1)
    out["sglob1"] = (j * 64 + s64 + 1).astype(np.int16).reshape(-1)
    out["cfold2"] = (j * TS - j * 128 + 0 * s64).astype(np.float32).reshape(-1)
    out["cwrap"] = (j * TS + 100.5 + 0 * s64).astype(np.float32).reshape(-1)
    return {k: np.tile(v[None, :], (128, 1)).copy() for k, v in out.items()}


def build_integration(nc, tc, pool, x0c, v0c, A, cvec, H):
    """100-step loop -> H [128,101,48] f32 SBUF."""
    v = nc.vector
    sq2s = OP_SQ2S()
    v16op = OP_V16()
    qw2 = OP_QW2N()
    qw1 = OP_QW1N()

    A = float(np.float32(A))
    c1f = float(np.float32(KC) * np.float32(A))
    c2f = float(np.float32(c1f) * np.float32(A))
    c1hf = float(np.float32(c1f) * np.float32(0.5))
    c2hf = float(np.float32(c2f) * np.float32(0.5))

    x0t = pool.tile([128, 48], f32)
    nc.sync.dma_start(x0t[:, :], x0c[:, :])
    u0 = pool.tile([128, 48], f32)
    nc.sync.dma_start(u0[:, :], v0c[:, :])
    v.tensor_scalar_mul(u0[:, :], u0[:, :], float(DT))  # u0 = dt*v0

    H3 = H  # [128, 101, 48]
    x03 = x0t[:, :].rearrange("p (a c) -> p a c", c=3)
    h03 = H3[:, 0, :].rearrange("p (a c) -> p a c", c=3)
    for ci in range(3):
        v.tensor_scalar_add(h03[:, :, ci], x03[:, :, ci], -float(np.float32(cvec[ci])))

    t12 = pool.tile([128, 16], f32, name="t12")
    v16t = pool.tile([128, 16], f32, name="v16t")
    q = pool.tile([128, 48], f32, name="qtile")

    def yv(t):  # [128, 16, 3] view of hist at step t
        return H3[:, t, :].rearrange("p (a c) -> p a c", c=3)

    def step(t, op, c1x, c2x):
        y3 = yv(t)
        v._custom_dve(sq2s, out=t12[:, :], in0=y3[:, :, 0], in1=y3[:, :, 1],
                      s0=ALPHA)
        v._custom_dve(v16op, out=v16t[:, :], in0=y3[:, :, 2], in1=t12[:, :],
                      s0=ALPHA)
        gb = v16t[:, :].rearrange("p (a o) -> p a o", o=1).to_broadcast(
            [128, 16, 3])
        v._custom_dve(op, out=q[:, :].rearrange("p (a c) -> p a c", c=3),
                      in0=y3, in1=gb, s0=c2x, s1=c1x)

    # Pure-DVE serial chain: run inside a critical section so there are no
    # per-instruction semaphores (in-order engine execution is sufficient).
    with tc.tile_critical(sync_engine=mybir.EngineType.DVE, name="evo"):
        step(0, qw1, c1hf, c2hf)                     # y1 = W1*y0 + dt*v0
        v.tensor_tensor(H3[:, 1, :], q[:, :], u0[:, :], ALU.add)
        for t in range(1, N_STEPS):                  # y_{t+1} = W*y_t - y_{t-1}
            step(t, qw2, c1f, c2f)
            v.tensor_tensor(H3[:, t + 1, :], q[:, :],
                            H3[:, t - 1, :], ALU.subtract)


def build_sampling(nc, tc, pool, H, zc, consts_dram, cvec, out_dram):
    """H: [128,101,48] SBUF f32; zc: DRAM [128,1024]; out_dram [128,3072]."""
    v = nc.vector
    s = nc.scalar
    g = nc.gpsimd
    sq2 = OP_SQ2()
    sqa = OP_SQA()
    msub1 = OP_MUL_SUB1()
    aff = OP_AFF2()

    zt = pool.tile([128, 1024], f32)
    nc.sync.dma_start(zt[:, :], zc[:, :])
    C = {}
    for name, dt_, n in CONST_SPECS:
        C[name] = pool.tile([128, n], _DTMAP[dt_], name="c_" + name)
        nc.sync.dma_start(C[name][:, :], consts_dram[name][:, :])

    # ====== z-side chain: all on Pool/ACT, overlaps the integration loop ====
    nkZ = pool.tile([128, 1024], f32, name="nkZ")
    U = pool.tile([128, BSP], i16, name="U")
    zctx = tc.tile_pool(name="zscr", bufs=1)
    zp = zctx.__enter__()
    bzr = zp.tile([128, 1024], i16, name="bzr")
    v.tensor_scalar(bzr[:, :], zt[:, :], 64.0, scalar2=-0.499,
                    op0=ALU.mult, op1=ALU.add)          # round -> floor(z*64)
    bzh = zp.tile([128, 1024], f16, name="bzh")
    v.tensor_copy(bzh[:, :], bzr[:, :])
    posZ1 = zp.tile([128, 1024], f16, name="posZ1")
    v.tensor_tensor(posZ1[:, :], bzh[:, :], C["cboffZ1"][:, :], ALU.add)
    kpZ = zp.tile([128, 16, 64], f32, name="kpZ")
    bz3 = bzr[:, :].rearrange("p (a b) -> p a b", b=64)
    v.tensor_tensor(kpZ[:, :, 0:63], bz3[:, :, 1:64], bz3[:, :, 0:63], ALU.is_gt)
    v.memset(kpZ[:, :, 63:64], 1.0)
    kpZf = kpZ[:, :, :].rearrange("p a b -> p (a b)")
    s.activation(nkZ[:, :], kpZf, AF.Copy, bias=1.0, scale=-1.0)
    idxZ = zp.tile([128, 1024], i16, name="idxZ")
    v._custom_dve(msub1, out=idxZ[:, :], in0=kpZf, in1=posZ1[:, :])
    g.local_scatter(U[:, :], C["sglob1"][:, :], idxZ[:, :],
                    channels=128, num_elems=BSP, num_idxs=1024)
    v.tensor_scalar_add(U[:, :], U[:, :], -1.0)          # U-1 (in-place)
    zctx.__exit__(None, None, None)

    # ================= T-side prep (after integration) ======================
    H3f = H[:, :, :].rearrange("p a (j c) -> p a j c", c=3)
    lctx = tc.tile_pool(name="lutscr", bufs=1)
    lp = lctx.__enter__()
    dch = []
    for ci in range(3):
        t_ = pool.tile([128, 16, TS], f16, name=f"dch{ci}")
        v.memset(t_[:, :, 100:102], 0.0)
        v.tensor_tensor(t_[:, :, 0:100].rearrange("p a b -> p b a"),
                        H3f[:, 1:101, :, ci], H3f[:, 0:100, :, ci], ALU.subtract)
        v.tensor_copy(t_[:, :, 100:101], t_[:, :, 99:100])   # dup-last-delta
        dch.append(t_)
    t2 = lp.tile([128, 16, 100], f32, name="t2scr")
    v._custom_dve(sq2, out=t2[:, :, :],
                  in0=dch[0][:, :, 0:100], in1=dch[1][:, :, 0:100])
    d2e = pool.tile([128, 16, TS], f32, name="d2e")
    v.memset(d2e[:, :, 0:1], 0.0)
    v.memset(d2e[:, :, 101:102], 0.0)
    v._custom_dve(sqa, out=d2e[:, :, 1:101],
                  in0=dch[2][:, :, 0:100], in1=t2[:, :, :])
    s.activation(d2e[:, :, 1:101], d2e[:, :, 1:101], AF.Sqrt)
    mks = lp.tile([128, 16, TS], f32, name="mks")
    v.memset(mks[:, :, :], 1.0)
    v.memset(mks[:, :, 0:1], 0.0)
    d2f = d2e[:, :, :].rearrange("p a b -> p (a b)")
    v.tensor_tensor_scan(d2f, mks[:, :, :].rearrange("p a b -> p (a b)"),
                         d2f, 0.0, ALU.mult, ALU.add)    # in-place cumsum -> D
    Dflat = d2f
    # fp16 payload channels: D and y (on ACT, off critical path)
    Dch = pool.tile([128, 16, TS], f16, name="Dch")
    s.activation(Dch[:, :, :].rearrange("p a b -> p (a b)"), Dflat, AF.Copy)
    ych = []
    for ci in range(3):
        t_ = pool.tile([128, 16, TS], f16, name=f"ych{ci}")
        v.memset(t_[:, :, 101:102], 0.0)
        src = H[:, :, :].rearrange("p a (j c) -> p j a c", c=3)[:, :, :, ci]
        s.activation(t_[:, :, 0:101], src, AF.Copy)
        ych.append(t_)

    # ================= bucket LUT -> cnt0 -> key1p ==========================
    btm = lp.tile([128, NTS], f32, name="btm")
    v.tensor_scalar(btm[:, :], Dflat, 64.0, scalar2=123.3, op0=ALU.mult, op1=ALU.min)
    posT = lp.tile([128, NTS], i16, name="posT")
    v.tensor_tensor(posT[:, :], btm[:, :], C["cboffT"][:, :], ALU.add)
    Gar = lp.tile([128, BSP], i16, name="Gar")
    g.local_scatter(Gar[:, :], C["gvals"][:, :], posT[:, :],
                    channels=128, num_elems=BSP, num_idxs=NTS)
    Gf = lp.tile([128, BSP], i16, name="Gf")
    v.tensor_tensor_scan(Gf[:, :], Gar[:, :], Gar[:, :], 0.0, ALU.max, ALU.max)
    cnt0r = lp.tile([128, 1024], i16, name="cnt0r")
    g.local_scatter(cnt0r[:, :], Gf[:, :], U[:, :],
                    channels=128, num_elems=1024, num_idxs=BSP)
    key1p = pool.tile([128, 1024], f32, name="key1p")
    v.tensor_tensor_scan(key1p[:, ::-1], nkZ[:, ::-1], cnt0r[:, ::-1],
                         0.0, ALU.mult, ALU.add)         # backward fill = cnt0
    v.tensor_tensor(key1p[:, :], key1p[:, :], C["cfold2"][:, :], ALU.add)
    lctx.__exit__(None, None, None)

    # ================= SLOT build ===========================================
    kp1 = pool.tile([128, 16, 64], f32, name="kp1")
    k3 = key1p[:, :].rearrange("p (a b) -> p a b", b=64)
    v.tensor_tensor(kp1[:, :, 0:63], k3[:, :, 1:64], k3[:, :, 0:63], ALU.is_gt)
    v.memset(kp1[:, :, 63:64], 1.0)
    kp1f = kp1[:, :, :].rearrange("p a b -> p (a b)")
    nk1 = pool.tile([128, 1024], f32, name="nk1")
    s.activation(nk1[:, :], kp1f, AF.Copy, bias=1.0, scale=-1.0)
    idxs = pool.tile([128, 1024], i16, name="idxs")
    v._custom_dve(msub1, out=idxs[:, :], in0=kp1f, in1=key1p[:, :])
    SLOT = pool.tile([128, NTS], i16, name="SLOT")
    g.local_scatter(SLOT[:, :], C["sglob1"][:, :], idxs[:, :],
                    channels=128, num_elems=NTS, num_idxs=1024)
    v.tensor_scalar_add(SLOT[:, :], SLOT[:, :], -1.0)    # in-place: SLOT-1

    # ================= payload delivery (7 fp16 channels) ===================
    dctx = tc.tile_pool(name="dscr", bufs=1)
    dp = dctx.__enter__()
    rawtags = ["rawA", "rawB", "rawC"]

    def deliver(data_ap, name, k):
        raw = dp.tile([128, 1024], f16, name="raw_" + name, tag=rawtags[k % 3])
        g.local_scatter(raw[:, :], data_ap, SLOT[:, :],
                        channels=128, num_elems=1024, num_idxs=NTS)
        out_t = pool.tile([128, 1024], f16, name="smp_" + name)
        v.tensor_tensor_scan(out_t[:, ::-1], nk1[:, ::-1], raw[:, ::-1],
                             0.0, ALU.mult, ALU.add)
        return out_t

    Dsmp = deliver(Dch[:, :, :].rearrange("p a b -> p (a b)"), "D", 0)
    dsmp = [deliver(dch[ci][:, :, :].rearrange("p a b -> p (a b)"), f"d{ci}", 1 + ci)
            for ci in range(3)]
    ysmp = [deliver(ych[ci][:, :, :].rearrange("p a b -> p (a b)"), f"y{ci}", 4 + ci)
            for ci in range(3)]

    # ================= final math ===========================================
    val = pool.tile([128, 1024], f32, name="val")
    v.tensor_tensor(val[:, :], zt[:, :], Dsmp[:, :], ALU.subtract)
    geo = pool.tile([128, 1024], f32, name="geo")
    v.tensor_scalar(geo[:, :], val[:, :], 0.0, scalar2=None, op0=ALU.is_ge)
    v.tensor_tensor(key1p[:, :], key1p[:, :], C["cwrap"][:, :], ALU.is_gt)
    wrapm = pool.tile([128, 1024], i16, name="wrapm")
    v.tensor_tensor(wrapm[:, :], key1p[:, :], geo[:, :], ALU.mult)
    for ci in range(3):
        pt = dp.tile([128, 1024], f16, name=f"pt{ci}", tag="pt")
        yib = ych[ci][:, :, 0:1].to_broadcast([128, 16, 64])
        v.tensor_tensor(pt[:, :].rearrange("p (a b) -> p a b", b=64), yib,
                        ysmp[ci][:, :].rearrange("p (a b) -> p a b", b=64),
                        ALU.subtract)
        v.copy_predicated(dsmp[ci][:, :], wrapm[:, :], pt[:, :])
    msq = pool.tile([128, 1024], f32, name="msq")
    v._custom_dve(sq2, out=msq[:, :], in0=dsmp[0][:, :], in1=dsmp[1][:, :])
    v._custom_dve(sqa, out=msq[:, :], in0=dsmp[2][:, :], in1=msq[:, :])
    inv = pool.tile([128, 1024], f32, name="inv")
    scr = pool.tile([128, 1024], f32, name="scr_inv")
    v.reciprocal_approx_accurate(inv[:, :], msq[:, :], scr[:, :])
    s.activation(inv[:, :], inv[:, :], AF.Sqrt)          # in-place rsqrt
    v.tensor_tensor(val[:, :], val[:, :], inv[:, :], ALU.mult)  # sc in-place
    out3 = pool.tile([128, 3072], f32, name="out3")
    o3 = out3[:, :].rearrange("p (s c) -> p s c", c=3)
    for ci in range(3):
        t_ = dp.tile([128, 1024], f32, name=f"sm{ci}", tag="sm")
        v.tensor_tensor(t_[:, :], val[:, :], dsmp[ci][:, :], ALU.mult)
        v._custom_dve(aff, out=o3[:, :, ci], in0=t_[:, :], in1=ysmp[ci][:, :],
                      s0=1.0, s1=float(np.float32(cvec[ci])))
    dctx.__exit__(None, None, None)
    nc.sync.dma_start(out_dram[:, :], out3[:, :])


# ---------------------------------------------------------------------------
_BUILD_CACHE = {}


def _build(A, cvec, n_cores=8):
    key = (float(np.float32(A)), tuple(float(np.float32(x)) for x in cvec))
    if key in _BUILD_CACHE:
        return _BUILD_CACHE[key]
    nc = bacc.Bacc("TRN2", target_bir_lowering=False, debug=False,
                   num_devices=n_cores)
    x0c = nc.dram_tensor("x0c", [128, 48], f32, kind="ExternalInput")
    v0c = nc.dram_tensor("v0c", [128, 48], f32, kind="ExternalInput")
    zc = nc.dram_tensor("zc", [128, 1024], f32, kind="ExternalInput")
    cdr = {}
    for name, dt_, n in CONST_SPECS:
        cdr[name] = nc.dram_tensor("cst_" + name, [128, n], _DTMAP[dt_],
                                   kind="ExternalInput")
    Oout = nc.dram_tensor("Oout", [128, 3072], f32, kind="ExternalOutput")
    with TileContext(nc) as tc:
        with tc.tile_pool(name="pp", bufs=1) as pool:
            H = pool.tile([128, 101, 48], f32)
            build_sampling_pre = None
            build_integration(nc, tc, pool, x0c, v0c, A, cvec, H)
            build_sampling(nc, tc, pool, H, zc, cdr, cvec, Oout)
    nc.compile()
    _BUILD_CACHE[key] = nc
    return nc


def kernel(x0, v0, z_vals, ior_center, ior_amp):
    """Full inputs -> full output [16384, 64, 3] float32."""
    x0 = np.ascontiguousarray(np.asarray(x0, np.float32))
    v0 = np.ascontiguousarray(np.asarray(v0, np.float32))
    z = np.ascontiguousarray(np.asarray(z_vals, np.float32)).reshape(16384, 64)
    c = np.asarray(ior_center, np.float32).reshape(3)
    A = float(np.asarray(ior_amp, np.float32).reshape(1)[0])
    n_cores = 8
    nc = _build(A, [float(c[0]), float(c[1]), float(c[2])], n_cores)
    cst = host_consts()
    in_maps = []
    for core in range(n_cores):
        sl = slice(core * 2048, (core + 1) * 2048)
        m = {"x0c": x0[sl].reshape(128, 48).copy(),
             "v0c": v0[sl].reshape(128, 48).copy(),
             "zc": z[sl].reshape(128, 1024).copy()}
        m.update({"cst_" + k: v for k, v in cst.items()})
        in_maps.append(m)
    res = run_bass_kernel_spmd(nc, in_maps, core_ids=list(range(n_cores)))
    out = np.empty((16384, 64, 3), np.float32)
    for core in range(n_cores):
        sl = slice(core * 2048, (core + 1) * 2048)
        out[sl] = res.results[core]["Oout"].reshape(2048, 64, 3)
    return out
